# revision 2
# baseline (speedup 1.0000x reference)
"""GCN (3-layer EnergyFlowGNN) Trainium2 Bass kernel, 8-core SPMD — v2.

Node-sharded pull design, rebuilt around trace findings from v1:
  - Gathers run on 4 SWDGE queues (queue = src-chunk), overlapping Q7
    descriptor generation across core pairs (~2.8 ns/edge vs 8 with 1 queue).
  - The symmetric norm dis[src]*dis[dst] is factorized away from the inner
    loop: dis_src is folded into the gather tables (xpad = dis*x, T2 =
    dis*h1, T3 = dis*s2 replicated), dis_dst is applied in the epilogue as
    per-partition ACT scales (relu commutes with a positive diagonal).
    The scatter one-hot m2w is therefore a PURE one-hot built by a single
    batched broadcast-AP is_equal per (pass, chunk) on DVE (bf16 out).
  - Scatter matmul: lhsT = one-hot [128 slots, 128 dst] (stationary), rhs =
    gathered messages [128 slots, CH] (moving), acc [128 dst, CH] in PSUM.
  - All tables are [N, 128] bf16 rows (256B gather elem, the HW minimum);
    T3 stores dis*s2 row-replicated so layer 3 needs no extraction.
  - Epilogues run on the otherwise idle Scalar (ACT) engine.
"""
import sys, os
sys.path.insert(0, "/opt/trn_rl_repo")
import numpy as np

import concourse.bacc as bacc
import concourse.mybir as mybir
import concourse.tile as tile
from concourse.tile import add_dep_helper
from concourse.bass_utils import run_bass_kernel_spmd

N_NODES = int(os.environ.get("KN", "100000"))
N_EDGES = 3200000
NF = 5
H = 64
NCORES = 8
NPC = N_NODES // NCORES          # nodes per core
NCHUNK = 4
CHUNK = N_NODES // NCHUNK        # table rows per chunk view (int16-safe)
PASSW = int(os.environ.get("KPASSW", "4"))   # dst windows per gather pass
TW = 128                         # table row width (bf16) = 256B gather elem

_prog_cache = {}


def _round128(x):
    return (x + 127) & ~127


def _bf16(a):
    """float32 ndarray -> bfloat16 ndarray (round to nearest even)."""
    import ml_dtypes
    return np.asarray(a, np.float32).astype(ml_dtypes.bfloat16)


def _prep(x, edge_index, W3=None):
    """Host-side sharding/layout. Returns per-core input maps + static meta.

    Self-loop edges are NOT placed in the gather streams: their contribution
    (identity matmul on a sequentially-loaded table block) is added on-device.
    Degrees still include the self-loops (GCN adds them before normalizing).
    """
    src = np.asarray(edge_index[0], np.int64)
    dst = np.asarray(edge_index[1], np.int64)
    deg = (np.bincount(dst, minlength=N_NODES) + 1).astype(np.float64)
    dis = np.where(deg > 0, 1.0 / np.sqrt(deg), 0.0).astype(np.float32)

    core = dst // NPC
    wloc = (dst - core * NPC) // 128
    g = src // CHUNK
    NW = (NPC + 127) // 128

    order = np.lexsort((g, wloc, core))
    src_o, dst_o, core_o, w_o, g_o = (
        a[order] for a in (src, dst, core, wloc, g))

    seg_cnt = np.zeros((NCORES, NW, NCHUNK), np.int64)
    np.add.at(seg_cnt, (core_o, w_o, g_o), 1)
    SEG = _round128(seg_cnt.max(axis=0))          # [NW, NCHUNK]
    Tg = SEG.sum(axis=0)                          # slots per chunk stream

    Tmax = int(Tg.max())
    idx16 = np.zeros((NCORES, NCHUNK, Tmax), np.int16)
    dstrel = np.full((NCORES, NCHUNK, Tmax), 255.0, np.float32)

    seg_starts = np.zeros((NW, NCHUNK), np.int64)
    segoff = np.zeros(NCHUNK, np.int64)
    for w in range(NW):
        for gg in range(NCHUNK):
            seg_starts[w, gg] = segoff[gg]
            segoff[gg] += SEG[w, gg]

    base = np.searchsorted(core_o, np.arange(NCORES))
    end = np.searchsorted(core_o, np.arange(NCORES), side="right")
    for c in range(NCORES):
        s_c = src_o[base[c]:end[c]]
        d_c = dst_o[base[c]:end[c]]
        w_c = w_o[base[c]:end[c]]
        g_c = g_o[base[c]:end[c]]
        key = w_c * NCHUNK + g_c
        bounds = np.searchsorted(key, np.arange(NW * NCHUNK + 1))
        for w in range(NW):
            for gg in range(NCHUNK):
                lo, hi = bounds[w * NCHUNK + gg], bounds[w * NCHUNK + gg + 1]
                n = hi - lo
                o = seg_starts[w, gg]
                idx16[c, gg, o:o + n] = (s_c[lo:hi] - gg * CHUNK).astype(np.int16)
                dstrel[c, gg, o:o + n] = ((d_c[lo:hi] - c * NPC) % 128).astype(np.float32)

    def wrap16(a):  # [T] -> [128, T//16]
        t = a.reshape(-1, 16).T
        return np.tile(t, (8, 1)).copy()

    def colmaj(a):  # [T] -> [128, T//128]
        return np.ascontiguousarray(a.reshape(-1, 128).T)

    xpad = np.zeros((N_NODES, TW), np.float32)
    xpad[:, :NF] = x * dis[:, None]
    xpad_b = _bf16(xpad)
    iota = np.tile(np.arange(128, dtype=np.float32)[None, :], (128, 1))
    ident = np.eye(128, dtype=np.float32)

    in_maps = []
    for c in range(NCORES):
        dd = np.zeros(NW * 128, np.float32)
        dd[:NPC] = dis[c * NPC:(c + 1) * NPC]
        m = {"xpad": xpad_b, "xloc": xpad_b[c * NPC:(c + 1) * NPC],
             "iota": iota, "ident": _bf16(ident),
             "onesr": _bf16(np.ones((1, 128), np.float32)),
             "disd": colmaj(dd)}
        for gg in range(NCHUNK):
            m[f"idx_{gg}"] = wrap16(idx16[c, gg, :Tg[gg]])
            m[f"dstrel_{gg}"] = colmaj(dstrel[c, gg, :Tg[gg]])
        in_maps.append(m)
    meta = (tuple(map(tuple, SEG)), tuple(int(t) for t in Tg))
    return in_maps, meta


def _build(meta):
    SEG = np.array(meta[0])        # [NW, NCHUNK]
    Tg = list(meta[1])
    NW = SEG.shape[0]
    f32 = mybir.dt.float32
    bf16 = mybir.dt.bfloat16
    i16 = mybir.dt.int16
    nc = bacc.Bacc("TRN2", target_bir_lowering=False, debug=False,
                   num_devices=NCORES, num_swdge_queues=4)

    xpad = nc.dram_tensor("xpad", [N_NODES, TW], bf16, kind="ExternalInput")
    xloc = nc.dram_tensor("xloc", [NPC, TW], bf16, kind="ExternalInput")
    iota_in = nc.dram_tensor("iota", [128, 128], f32, kind="ExternalInput")
    id_in = nc.dram_tensor("ident", [128, 128], bf16, kind="ExternalInput")
    ones_in = nc.dram_tensor("onesr", [1, 128], bf16, kind="ExternalInput")
    disd_in = nc.dram_tensor("disd", [128, NW], f32, kind="ExternalInput")
    W1_in = nc.dram_tensor("W1", [NF, H], f32, kind="ExternalInput")
    b1_in = nc.dram_tensor("b1c", [H, 1], f32, kind="ExternalInput")
    W2_in = nc.dram_tensor("W2", [H, H], f32, kind="ExternalInput")
    b2_in = nc.dram_tensor("b2c", [H, 1], f32, kind="ExternalInput")
    W3_in = nc.dram_tensor("W3", [H, 1], f32, kind="ExternalInput")
    b3_in = nc.dram_tensor("b3r", [128, 1], f32, kind="ExternalInput")
    ins_g = {}
    for gg in range(NCHUNK):
        ins_g[("i", gg)] = nc.dram_tensor(f"idx_{gg}", [128, Tg[gg] // 16],
                                          i16, kind="ExternalInput")
        ins_g[("d", gg)] = nc.dram_tensor(f"dstrel_{gg}", [128, Tg[gg] // 128],
                                          f32, kind="ExternalInput")
    out = nc.dram_tensor("out", [NPC, 1], f32, kind="ExternalOutput")
    T2loc = nc.dram_tensor("T2loc", [NPC, TW], bf16)
    T3loc = nc.dram_tensor("T3loc", [NPC, TW], bf16)
    T2 = nc.dram_tensor("T2", [N_NODES, TW], bf16, addr_space="Shared")
    T3 = nc.dram_tensor("T3", [N_NODES, TW], bf16, addr_space="Shared")

    NPASS = (NW + PASSW - 1) // PASSW
    seg_off = np.zeros((NW, NCHUNK), np.int64)
    segoff = np.zeros(NCHUNK, np.int64)
    for w in range(NW):
        for gg in range(NCHUNK):
            seg_off[w, gg] = segoff[gg]
            segoff[gg] += SEG[w, gg]

    from contextlib import ExitStack
    _gstk = ExitStack()
    with tile.TileContext(nc) as tc:
        cpool = _gstk.enter_context(tc.tile_pool(name="const", bufs=1))
        iota_t = cpool.tile([128, 128], f32); nc.sync.dma_start(out=iota_t[:], in_=iota_in[:])
        id_t = cpool.tile([128, 128], bf16); nc.sync.dma_start(out=id_t[:], in_=id_in[:])
        ones_t = cpool.tile([1, 128], bf16); nc.sync.dma_start(out=ones_t[:], in_=ones_in[:])
        disd_t = cpool.tile([128, NW], f32); nc.sync.dma_start(out=disd_t[:], in_=disd_in[:])
        W1f = cpool.tile([NF, H], f32); nc.sync.dma_start(out=W1f[:], in_=W1_in[:])
        W2f = cpool.tile([H, H], f32); nc.sync.dma_start(out=W2f[:], in_=W2_in[:])
        W3f = cpool.tile([H, 1], f32); nc.sync.dma_start(out=W3f[:], in_=W3_in[:])
        b1_t = cpool.tile([H, 1], f32); nc.sync.dma_start(out=b1_t[:], in_=b1_in[:])
        b2_t = cpool.tile([H, 1], f32); nc.sync.dma_start(out=b2_t[:], in_=b2_in[:])
        b3_t = cpool.tile([128, 1], f32); nc.sync.dma_start(out=b3_t[:], in_=b3_in[:])
        W1b = cpool.tile([NF, H], bf16); nc.vector.tensor_copy(out=W1b[:], in_=W1f[:])
        W2b = cpool.tile([H, H], bf16); nc.vector.tensor_copy(out=W2b[:], in_=W2f[:])
        W3b = cpool.tile([H, 1], bf16); nc.vector.tensor_copy(out=W3b[:], in_=W3f[:])

        all_gathers = []
        Copy = mybir.ActivationFunctionType.Copy
        Relu = mybir.ActivationFunctionType.Relu
        Ident = mybir.ActivationFunctionType.Identity

        def run_layer(layer, table_views, loc_tab, table_dep=None):
            stk = ExitStack()
            ipool = stk.enter_context(tc.tile_pool(name=f"ix{layer}", bufs=2))
            mpool = stk.enter_context(tc.tile_pool(name=f"msg{layer}", bufs=2))
            wpool = stk.enter_context(tc.tile_pool(name=f"oh{layer}", bufs=1))
            ppool = stk.enter_context(tc.tile_pool(name=f"ps{layer}", bufs=PASSW,
                                                   space="PSUM"))
            gpool = stk.enter_context(tc.tile_pool(name=f"ep{layer}", bufs=1,
                                                   space="PSUM"))
            spool = stk.enter_context(tc.tile_pool(name=f"sb{layer}", bufs=3))
            wdmas = []
            CH = NF if layer == 1 else (H if layer == 2 else 1)
            for p in range(NPASS):
                ws = list(range(p * PASSW, min((p + 1) * PASSW, NW)))
                bufs, offs = {}, {}
                for gg in range(NCHUNK):
                    n = int(SEG[ws, gg].sum())
                    if n == 0:
                        continue
                    c0 = int(seg_off[ws[0], gg])
                    K = n // 128
                    it = ipool.tile([128, max(n, 128) // 16], i16, tag=f"it{gg}")
                    ld = nc.sync.dma_start(
                        out=it[:, :n // 16],
                        in_=ins_g[("i", gg)][:, c0 // 16:(c0 + n) // 16])
                    mt = mpool.tile([128, max(n, 128)], bf16, tag=f"mt{gg}")
                    gv = mt[:, :n].rearrange("p (k c) -> p k c", k=K, c=TW)
                    gth = nc.gpsimd.dma_gather(
                        out_ap=gv, in_ap=table_views[gg], idxs_ap=it[:, :n // 16],
                        num_idxs=n, num_idxs_reg=n, elem_size=TW,
                        single_packet=False, queue_num=gg)
                    add_dep_helper(gth.ins, ld.ins, True, "gather reads idx")
                    if table_dep is not None:
                        add_dep_helper(gth.ins, table_dep.ins, True,
                                       "gather reads table")
                    all_gathers.append(gth)
                    # batched pure one-hot: m2w = (iota == dstrel), bf16 out
                    dre = ipool.tile([128, max(K, 1)], f32, tag=f"dre{gg}")
                    nc.sync.dma_start(out=dre[:, :K],
                                      in_=ins_g[("d", gg)][:, c0 // 128:c0 // 128 + K])
                    m2w = wpool.tile([128, max(n, 128)], bf16, tag=f"m2w{gg}")
                    m2wv = m2w[:, :n].rearrange("p (k c) -> p k c", k=K, c=128)
                    iota_b = iota_t[:].unsqueeze(1).to_broadcast([128, K, 128])
                    dre_b = dre[:, :K].unsqueeze(2).to_broadcast([128, K, 128])
                    nc.vector.tensor_tensor(out=m2wv, in0=iota_b, in1=dre_b,
                                            op=mybir.AluOpType.is_equal)
                    bufs[gg] = (mt, m2w, gth)
                    offs[gg] = c0
                for w in ws:
                    ngrp = int(SEG[w].sum()) // 128
                    acc = ppool.tile([128, CH], f32, tag="acc", space="PSUM")
                    wn = min(128, NPC - w * 128)
                    # self-loop contribution: identity matmul on the core's
                    # own (sequentially loaded) table rows for this window
                    tb = ipool.tile([128, TW], bf16, tag="tb")
                    tl = nc.sync.dma_start(out=tb[:wn, :],
                                           in_=loc_tab[w * 128:w * 128 + wn, :])
                    if table_dep is not None:
                        add_dep_helper(tl.ins, table_dep.ins, True,
                                       "tblk reads local table")
                    mmi = nc.tensor.matmul(
                        out=acc[:], lhsT=id_t[:wn, :], rhs=tb[:wn, :CH],
                        start=True, stop=(ngrp == 0))
                    add_dep_helper(mmi.ins, tl.ins, True, "id mm reads tblk")
                    gi = 0
                    for gg in range(NCHUNK):
                        nseg = int(SEG[w, gg])
                        if nseg == 0:
                            continue
                        mt, m2w, gth = bufs[gg]
                        for k in range(nseg // 128):
                            kk = (int(seg_off[w, gg]) - offs[gg]) // 128 + k
                            mm = nc.tensor.matmul(
                                out=acc[:],
                                lhsT=m2w[:, kk * 128:(kk + 1) * 128],
                                rhs=mt[:, kk * TW:kk * TW + CH],
                                start=False, stop=(gi == ngrp - 1))
                            add_dep_helper(mm.ins, gth.ins, True, "mm reads msg")
                            gi += 1
                    dw = disd_t[:, w:w + 1]
                    if layer == 3:
                        o3 = spool.tile([128, 1], f32, tag="o3")
                        nc.scalar.activation(o3[:], acc[:], Ident,
                                             bias=b3_t[:], scale=dw)
                        wdmas.append(nc.sync.dma_start(
                            out=out[w * 128:w * 128 + wn, :], in_=o3[:wn, :]))
                        continue
                    # layers 1 & 2: as = dis_dst * acc   [128, CH] bf16
                    asb = spool.tile([128, CH], bf16, tag="asb")
                    nc.scalar.activation(asb[:], acc[:], Copy, scale=dw)
                    tp = gpool.tile([CH, 128], bf16, tag="tp", space="PSUM")
                    nc.tensor.transpose(out=tp[:], in_=asb[:],
                                        identity=id_t[:])
                    tps = spool.tile([CH, 128], bf16, tag="tps")
                    nc.scalar.activation(tps[:], tp[:], Copy)
                    Wb = W1b if layer == 1 else W2b
                    bb = b1_t if layer == 1 else b2_t
                    hT = gpool.tile([H, 128], f32, tag="hT", space="PSUM")
                    nc.tensor.matmul(out=hT[:], lhsT=Wb[:], rhs=tps[:],
                                     start=True, stop=True)
                    hTs = spool.tile([H, 128], bf16, tag="hTs")
                    nc.scalar.activation(hTs[:], hT[:], Relu, bias=bb[:])
                    if layer == 1:
                        tr = gpool.tile([128, H], bf16, tag="tr", space="PSUM")
                        nc.tensor.transpose(out=tr[:], in_=hTs[:],
                                            identity=id_t[:H, :H])
                        trs = spool.tile([128, H], bf16, tag="trs")
                        nc.scalar.activation(trs[:], tr[:], Relu, scale=dw)
                        wdmas.append(nc.sync.dma_start(
                            out=T2loc[w * 128:w * 128 + wn, :H],
                            in_=trs[:wn, :]))
                    else:
                        s2p = gpool.tile([1, 128], f32, tag="s2p", space="PSUM")
                        nc.tensor.matmul(out=s2p[:], lhsT=W3b[:], rhs=hTs[:],
                                         start=True, stop=True)
                        s2s = spool.tile([1, 128], bf16, tag="s2s")
                        nc.scalar.activation(s2s[:], s2p[:], Copy)
                        rep = gpool.tile([128, 128], f32, tag="rep", space="PSUM")
                        nc.tensor.matmul(out=rep[:], lhsT=s2s[:], rhs=ones_t[:],
                                         start=True, stop=True)
                        reps = spool.tile([128, 128], bf16, tag="reps")
                        nc.scalar.activation(reps[:], rep[:], Ident, scale=dw)
                        wdmas.append(nc.sync.dma_start(
                            out=T3loc[w * 128:w * 128 + wn, :],
                            in_=reps[:wn, :]))
            stk.close()
            return wdmas

        tv1 = [xpad[gg * CHUNK:(gg + 1) * CHUNK, :] for gg in range(NCHUNK)]
        wd1 = run_layer(1, tv1, xloc)
        coll1 = nc.gpsimd.collective_compute(
            "AllGather", mybir.AluOpType.bypass,
            replica_groups=[list(range(NCORES))],
            ins=[T2loc[:, :]], outs=[T2[:, :]])
        for d in wd1:
            add_dep_helper(coll1.ins, d.ins, True, "allgather waits T2loc")
        tv2 = [T2[gg * CHUNK:(gg + 1) * CHUNK, :] for gg in range(NCHUNK)]
        wd2 = run_layer(2, tv2, T2loc, table_dep=coll1)
        coll2 = nc.gpsimd.collective_compute(
            "AllGather", mybir.AluOpType.bypass,
            replica_groups=[list(range(NCORES))],
            ins=[T3loc[:, :]], outs=[T3[:, :]])
        for d in wd2:
            add_dep_helper(coll2.ins, d.ins, True, "allgather waits T3loc")
        tv3 = [T3[gg * CHUNK:(gg + 1) * CHUNK, :] for gg in range(NCHUNK)]
        run_layer(3, tv3, T3loc, table_dep=coll2)

        guard = cpool.tile([128, H], bf16, tag="guard")
        pw = nc.gpsimd.dma_start(out=guard[:], in_=T2[:128, :H])
        for gth in all_gathers[-8:]:
            add_dep_helper(pw.ins, gth.ins, True, "pool drain guard")
        _gstk.close()
    nc.compile()
    return nc


def kernel(x, edge_index, W1, b1, W2, b2, W3, b3):
    x = np.asarray(x, np.float32)
    edge_index = np.asarray(edge_index, np.int64)
    in_maps, meta = _prep(x, edge_index, W3)
    key = hash(meta)
    if key not in _prog_cache:
        _prog_cache[key] = _build(meta)
    nc = _prog_cache[key]
    shared = {
        "W1": np.asarray(W1, np.float32),
        "b1c": np.asarray(b1, np.float32).reshape(H, 1),
        "W2": np.asarray(W2, np.float32),
        "b2c": np.asarray(b2, np.float32).reshape(H, 1),
        "W3": np.asarray(W3, np.float32).reshape(H, 1),
        "b3r": np.full((128, 1), np.float32(np.asarray(b3).reshape(-1)[0])),
    }
    for m in in_maps:
        m.update(shared)
    res = run_bass_kernel_spmd(nc, in_maps, core_ids=list(range(NCORES)))
    outp = np.concatenate([res.results[c]["out"].reshape(-1)
                           for c in range(NCORES)])
    return outp.reshape(N_NODES, 1).astype(np.float32)


# revision 3
# speedup vs baseline: 1.0899x; 1.0899x over previous
"""GCN (3-layer EnergyFlowGNN) Trainium2 Bass kernel, 8-core SPMD — v2.

Node-sharded pull design, rebuilt around trace findings from v1:
  - Gathers run on 4 SWDGE queues (queue = src-chunk), overlapping Q7
    descriptor generation across core pairs (~2.8 ns/edge vs 8 with 1 queue).
  - The symmetric norm dis[src]*dis[dst] is factorized away from the inner
    loop: dis_src is folded into the gather tables (xpad = dis*x, T2 =
    dis*h1, T3 = dis*s2 replicated), dis_dst is applied in the epilogue as
    per-partition ACT scales (relu commutes with a positive diagonal).
    The scatter one-hot m2w is therefore a PURE one-hot built by a single
    batched broadcast-AP is_equal per (pass, chunk) on DVE (bf16 out).
  - Scatter matmul: lhsT = one-hot [128 slots, 128 dst] (stationary), rhs =
    gathered messages [128 slots, CH] (moving), acc [128 dst, CH] in PSUM.
  - All tables are [N, 128] bf16 rows (256B gather elem, the HW minimum);
    T3 stores dis*s2 row-replicated so layer 3 needs no extraction.
  - Epilogues run on the otherwise idle Scalar (ACT) engine.
"""
import sys, os
sys.path.insert(0, "/opt/trn_rl_repo")
import numpy as np

import concourse.bacc as bacc
import concourse.mybir as mybir
import concourse.tile as tile
from concourse.tile import add_dep_helper
from concourse.bass_utils import run_bass_kernel_spmd

N_NODES = int(os.environ.get("KN", "100000"))
N_EDGES = 3200000
NF = 5
H = 64
NCORES = 8
NPC = N_NODES // NCORES          # nodes per core
NCHUNK = 4
CHUNK = N_NODES // NCHUNK        # table rows per chunk view (int16-safe)
PASSW = int(os.environ.get("KPASSW", "4"))   # dst windows per gather pass
TW = 128                         # table row width (bf16) = 256B gather elem

_prog_cache = {}


def _round128(x):
    return (x + 127) & ~127


def _bf16(a):
    """float32 ndarray -> bfloat16 ndarray (round to nearest even)."""
    import ml_dtypes
    return np.asarray(a, np.float32).astype(ml_dtypes.bfloat16)


def _prep(x, edge_index, W3=None):
    """Host-side sharding/layout. Returns per-core input maps + static meta.

    Self-loop edges are NOT placed in the gather streams: their contribution
    (identity matmul on a sequentially-loaded table block) is added on-device.
    Degrees still include the self-loops (GCN adds them before normalizing).
    """
    src = np.asarray(edge_index[0], np.int64)
    dst = np.asarray(edge_index[1], np.int64)
    deg = (np.bincount(dst, minlength=N_NODES) + 1).astype(np.float64)
    dis = np.where(deg > 0, 1.0 / np.sqrt(deg), 0.0).astype(np.float32)

    core = dst // NPC
    wloc = (dst - core * NPC) // 128
    g = src // CHUNK
    NW = (NPC + 127) // 128

    order = np.lexsort((g, wloc, core))
    src_o, dst_o, core_o, w_o, g_o = (
        a[order] for a in (src, dst, core, wloc, g))

    seg_cnt = np.zeros((NCORES, NW, NCHUNK), np.int64)
    np.add.at(seg_cnt, (core_o, w_o, g_o), 1)
    SEG = _round128(seg_cnt.max(axis=0))          # [NW, NCHUNK]
    Tg = SEG.sum(axis=0)                          # slots per chunk stream

    Tmax = int(Tg.max())
    idx16 = np.zeros((NCORES, NCHUNK, Tmax), np.int16)
    dstrel = np.full((NCORES, NCHUNK, Tmax), 255.0, np.float32)

    seg_starts = np.zeros((NW, NCHUNK), np.int64)
    segoff = np.zeros(NCHUNK, np.int64)
    for w in range(NW):
        for gg in range(NCHUNK):
            seg_starts[w, gg] = segoff[gg]
            segoff[gg] += SEG[w, gg]

    base = np.searchsorted(core_o, np.arange(NCORES))
    end = np.searchsorted(core_o, np.arange(NCORES), side="right")
    for c in range(NCORES):
        s_c = src_o[base[c]:end[c]]
        d_c = dst_o[base[c]:end[c]]
        w_c = w_o[base[c]:end[c]]
        g_c = g_o[base[c]:end[c]]
        key = w_c * NCHUNK + g_c
        bounds = np.searchsorted(key, np.arange(NW * NCHUNK + 1))
        for w in range(NW):
            for gg in range(NCHUNK):
                lo, hi = bounds[w * NCHUNK + gg], bounds[w * NCHUNK + gg + 1]
                n = hi - lo
                o = seg_starts[w, gg]
                idx16[c, gg, o:o + n] = (s_c[lo:hi] - gg * CHUNK).astype(np.int16)
                dstrel[c, gg, o:o + n] = ((d_c[lo:hi] - c * NPC) % 128).astype(np.float32)

    def wrap16(a):  # [T] -> [128, T//16]
        t = a.reshape(-1, 16).T
        return np.tile(t, (8, 1)).copy()

    def colmaj(a):  # [T] -> [128, T//128]
        return np.ascontiguousarray(a.reshape(-1, 128).T)

    xpad = np.zeros((N_NODES, TW), np.float32)
    xpad[:, :NF] = x * dis[:, None]
    xpad_b = _bf16(xpad)
    iota = np.tile(np.arange(128, dtype=np.float32)[None, :], (128, 1))
    ident = np.eye(128, dtype=np.float32)

    in_maps = []
    for c in range(NCORES):
        dd = np.zeros(NW * 128, np.float32)
        dd[:NPC] = dis[c * NPC:(c + 1) * NPC]
        m = {"xpad": xpad_b, "xloc": xpad_b[c * NPC:(c + 1) * NPC],
             "iota": iota, "ident": _bf16(ident),
             "onesr": _bf16(np.ones((1, 128), np.float32)),
             "disd": colmaj(dd)}
        for gg in range(NCHUNK):
            m[f"idx_{gg}"] = wrap16(idx16[c, gg, :Tg[gg]])
            m[f"dstrel_{gg}"] = colmaj(dstrel[c, gg, :Tg[gg]])
        in_maps.append(m)
    meta = (tuple(map(tuple, SEG)), tuple(int(t) for t in Tg))
    return in_maps, meta


def _build(meta):
    SEG = np.array(meta[0])        # [NW, NCHUNK]
    Tg = list(meta[1])
    NW = SEG.shape[0]
    f32 = mybir.dt.float32
    bf16 = mybir.dt.bfloat16
    i16 = mybir.dt.int16
    nc = bacc.Bacc("TRN2", target_bir_lowering=False, debug=False,
                   num_devices=NCORES, num_swdge_queues=4)

    xpad = nc.dram_tensor("xpad", [N_NODES, TW], bf16, kind="ExternalInput")
    xloc = nc.dram_tensor("xloc", [NPC, TW], bf16, kind="ExternalInput")
    iota_in = nc.dram_tensor("iota", [128, 128], f32, kind="ExternalInput")
    id_in = nc.dram_tensor("ident", [128, 128], bf16, kind="ExternalInput")
    ones_in = nc.dram_tensor("onesr", [1, 128], bf16, kind="ExternalInput")
    disd_in = nc.dram_tensor("disd", [128, NW], f32, kind="ExternalInput")
    W1_in = nc.dram_tensor("W1", [NF, H], f32, kind="ExternalInput")
    b1_in = nc.dram_tensor("b1c", [H, 1], f32, kind="ExternalInput")
    W2_in = nc.dram_tensor("W2", [H, H], f32, kind="ExternalInput")
    b2_in = nc.dram_tensor("b2c", [H, 1], f32, kind="ExternalInput")
    W3_in = nc.dram_tensor("W3", [H, 1], f32, kind="ExternalInput")
    b3_in = nc.dram_tensor("b3r", [128, 1], f32, kind="ExternalInput")
    ins_g = {}
    for gg in range(NCHUNK):
        ins_g[("i", gg)] = nc.dram_tensor(f"idx_{gg}", [128, Tg[gg] // 16],
                                          i16, kind="ExternalInput")
        ins_g[("d", gg)] = nc.dram_tensor(f"dstrel_{gg}", [128, Tg[gg] // 128],
                                          f32, kind="ExternalInput")
    out = nc.dram_tensor("out", [NPC, 1], f32, kind="ExternalOutput")
    T2loc = nc.dram_tensor("T2loc", [NPC, TW], bf16)
    T3loc = nc.dram_tensor("T3loc", [NPC, TW], bf16)
    T2 = nc.dram_tensor("T2", [N_NODES, TW], bf16, addr_space="Shared")
    T3 = nc.dram_tensor("T3", [N_NODES, TW], bf16, addr_space="Shared")

    NPASS = (NW + PASSW - 1) // PASSW
    seg_off = np.zeros((NW, NCHUNK), np.int64)
    segoff = np.zeros(NCHUNK, np.int64)
    for w in range(NW):
        for gg in range(NCHUNK):
            seg_off[w, gg] = segoff[gg]
            segoff[gg] += SEG[w, gg]

    from contextlib import ExitStack
    _gstk = ExitStack()
    with tile.TileContext(nc) as tc:
        cpool = _gstk.enter_context(tc.tile_pool(name="const", bufs=1))
        iota_t = cpool.tile([128, 128], f32); nc.sync.dma_start(out=iota_t[:], in_=iota_in[:])
        id_t = cpool.tile([128, 128], bf16); nc.sync.dma_start(out=id_t[:], in_=id_in[:])
        ones_t = cpool.tile([1, 128], bf16); nc.sync.dma_start(out=ones_t[:], in_=ones_in[:])
        disd_t = cpool.tile([128, NW], f32); nc.sync.dma_start(out=disd_t[:], in_=disd_in[:])
        W1f = cpool.tile([NF, H], f32); nc.sync.dma_start(out=W1f[:], in_=W1_in[:])
        W2f = cpool.tile([H, H], f32); nc.sync.dma_start(out=W2f[:], in_=W2_in[:])
        W3f = cpool.tile([H, 1], f32); nc.sync.dma_start(out=W3f[:], in_=W3_in[:])
        b1_t = cpool.tile([H, 1], f32); nc.sync.dma_start(out=b1_t[:], in_=b1_in[:])
        b2_t = cpool.tile([H, 1], f32); nc.sync.dma_start(out=b2_t[:], in_=b2_in[:])
        b3_t = cpool.tile([128, 1], f32); nc.sync.dma_start(out=b3_t[:], in_=b3_in[:])
        W1b = cpool.tile([NF, H], bf16); nc.vector.tensor_copy(out=W1b[:], in_=W1f[:])
        W2b = cpool.tile([H, H], bf16); nc.vector.tensor_copy(out=W2b[:], in_=W2f[:])
        W3b = cpool.tile([H, 1], bf16); nc.vector.tensor_copy(out=W3b[:], in_=W3f[:])

        all_gathers = []
        Copy = mybir.ActivationFunctionType.Copy
        Relu = mybir.ActivationFunctionType.Relu
        Ident = mybir.ActivationFunctionType.Identity

        def run_layer(layer, table_views, loc_tab, table_dep=None):
            stk = ExitStack()
            ipool = stk.enter_context(tc.tile_pool(name=f"ix{layer}", bufs=2))
            mpool = stk.enter_context(tc.tile_pool(name=f"msg{layer}", bufs=2))
            wpool = stk.enter_context(tc.tile_pool(name=f"oh{layer}", bufs=2))
            ppool = stk.enter_context(tc.tile_pool(name=f"ps{layer}", bufs=PASSW,
                                                   space="PSUM"))
            gpool = stk.enter_context(tc.tile_pool(name=f"ep{layer}", bufs=1,
                                                   space="PSUM"))
            spool = stk.enter_context(tc.tile_pool(name=f"sb{layer}", bufs=3))
            wdmas = []
            CH = NF if layer == 1 else (H if layer == 2 else 1)
            for p in range(NPASS):
                ws = list(range(p * PASSW, min((p + 1) * PASSW, NW)))
                bufs, offs = {}, {}
                for gg in range(NCHUNK):
                    n = int(SEG[ws, gg].sum())
                    if n == 0:
                        continue
                    c0 = int(seg_off[ws[0], gg])
                    K = n // 128
                    it = ipool.tile([128, max(n, 128) // 16], i16, tag=f"it{gg}")
                    ld = nc.sync.dma_start(
                        out=it[:, :n // 16],
                        in_=ins_g[("i", gg)][:, c0 // 16:(c0 + n) // 16])
                    mt = mpool.tile([128, max(n, 128)], bf16, tag=f"mt{gg}")
                    gv = mt[:, :n].rearrange("p (k c) -> p k c", k=K, c=TW)
                    gth = nc.gpsimd.dma_gather(
                        out_ap=gv, in_ap=table_views[gg], idxs_ap=it[:, :n // 16],
                        num_idxs=n, num_idxs_reg=n, elem_size=TW,
                        single_packet=False, queue_num=gg)
                    add_dep_helper(gth.ins, ld.ins, True, "gather reads idx")
                    if table_dep is not None:
                        add_dep_helper(gth.ins, table_dep.ins, True,
                                       "gather reads table")
                    all_gathers.append(gth)
                    # batched pure one-hot: m2w = (iota == dstrel), bf16 out
                    dre = ipool.tile([128, max(K, 1)], f32, tag=f"dre{gg}")
                    nc.sync.dma_start(out=dre[:, :K],
                                      in_=ins_g[("d", gg)][:, c0 // 128:c0 // 128 + K])
                    m2w = wpool.tile([128, max(n, 128)], bf16, tag=f"m2w{gg}")
                    m2wv = m2w[:, :n].rearrange("p (k c) -> p k c", k=K, c=128)
                    iota_b = iota_t[:].unsqueeze(1).to_broadcast([128, K, 128])
                    dre_b = dre[:, :K].unsqueeze(2).to_broadcast([128, K, 128])
                    nc.vector.tensor_tensor(out=m2wv, in0=iota_b, in1=dre_b,
                                            op=mybir.AluOpType.is_equal)
                    bufs[gg] = (mt, m2w, gth)
                    offs[gg] = c0
                for w in ws:
                    ngrp = int(SEG[w].sum()) // 128
                    acc = ppool.tile([128, CH], f32, tag="acc", space="PSUM")
                    wn = min(128, NPC - w * 128)
                    # self-loop contribution: identity matmul on the core's
                    # own (sequentially loaded) table rows for this window
                    tb = ipool.tile([128, TW], bf16, tag="tb")
                    tl = nc.sync.dma_start(out=tb[:wn, :],
                                           in_=loc_tab[w * 128:w * 128 + wn, :])
                    if table_dep is not None:
                        add_dep_helper(tl.ins, table_dep.ins, True,
                                       "tblk reads local table")
                    mmi = nc.tensor.matmul(
                        out=acc[:], lhsT=id_t[:wn, :], rhs=tb[:wn, :CH],
                        start=True, stop=(ngrp == 0))
                    add_dep_helper(mmi.ins, tl.ins, True, "id mm reads tblk")
                    gi = 0
                    for gg in range(NCHUNK):
                        nseg = int(SEG[w, gg])
                        if nseg == 0:
                            continue
                        mt, m2w, gth = bufs[gg]
                        for k in range(nseg // 128):
                            kk = (int(seg_off[w, gg]) - offs[gg]) // 128 + k
                            mm = nc.tensor.matmul(
                                out=acc[:],
                                lhsT=m2w[:, kk * 128:(kk + 1) * 128],
                                rhs=mt[:, kk * TW:kk * TW + CH],
                                start=False, stop=(gi == ngrp - 1))
                            add_dep_helper(mm.ins, gth.ins, True, "mm reads msg")
                            gi += 1
                    dw = disd_t[:, w:w + 1]
                    if layer == 3:
                        o3 = spool.tile([128, 1], f32, tag="o3")
                        nc.scalar.activation(o3[:], acc[:], Ident,
                                             bias=b3_t[:], scale=dw)
                        wdmas.append(nc.sync.dma_start(
                            out=out[w * 128:w * 128 + wn, :], in_=o3[:wn, :]))
                        continue
                    # layers 1 & 2: as = dis_dst * acc   [128, CH] bf16
                    asb = spool.tile([128, CH], bf16, tag="asb")
                    nc.scalar.activation(asb[:], acc[:], Copy, scale=dw)
                    tp = gpool.tile([CH, 128], bf16, tag="tp", space="PSUM")
                    nc.tensor.transpose(out=tp[:], in_=asb[:],
                                        identity=id_t[:])
                    tps = spool.tile([CH, 128], bf16, tag="tps")
                    nc.scalar.activation(tps[:], tp[:], Copy)
                    Wb = W1b if layer == 1 else W2b
                    bb = b1_t if layer == 1 else b2_t
                    hT = gpool.tile([H, 128], f32, tag="hT", space="PSUM")
                    nc.tensor.matmul(out=hT[:], lhsT=Wb[:], rhs=tps[:],
                                     start=True, stop=True)
                    hTs = spool.tile([H, 128], bf16, tag="hTs")
                    nc.scalar.activation(hTs[:], hT[:], Relu, bias=bb[:])
                    if layer == 1:
                        tr = gpool.tile([128, H], bf16, tag="tr", space="PSUM")
                        nc.tensor.transpose(out=tr[:], in_=hTs[:],
                                            identity=id_t[:H, :H])
                        trs = spool.tile([128, H], bf16, tag="trs")
                        nc.scalar.activation(trs[:], tr[:], Relu, scale=dw)
                        wdmas.append(nc.sync.dma_start(
                            out=T2loc[w * 128:w * 128 + wn, :H],
                            in_=trs[:wn, :]))
                    else:
                        s2p = gpool.tile([1, 128], f32, tag="s2p", space="PSUM")
                        nc.tensor.matmul(out=s2p[:], lhsT=W3b[:], rhs=hTs[:],
                                         start=True, stop=True)
                        s2s = spool.tile([1, 128], bf16, tag="s2s")
                        nc.scalar.activation(s2s[:], s2p[:], Copy)
                        rep = gpool.tile([128, 128], f32, tag="rep", space="PSUM")
                        nc.tensor.matmul(out=rep[:], lhsT=s2s[:], rhs=ones_t[:],
                                         start=True, stop=True)
                        reps = spool.tile([128, 128], bf16, tag="reps")
                        nc.scalar.activation(reps[:], rep[:], Ident, scale=dw)
                        wdmas.append(nc.sync.dma_start(
                            out=T3loc[w * 128:w * 128 + wn, :],
                            in_=reps[:wn, :]))
            stk.close()
            return wdmas

        tv1 = [xpad[gg * CHUNK:(gg + 1) * CHUNK, :] for gg in range(NCHUNK)]
        wd1 = run_layer(1, tv1, xloc)
        coll1 = nc.gpsimd.collective_compute(
            "AllGather", mybir.AluOpType.bypass,
            replica_groups=[list(range(NCORES))],
            ins=[T2loc[:, :]], outs=[T2[:, :]])
        for d in wd1:
            add_dep_helper(coll1.ins, d.ins, True, "allgather waits T2loc")
        tv2 = [T2[gg * CHUNK:(gg + 1) * CHUNK, :] for gg in range(NCHUNK)]
        wd2 = run_layer(2, tv2, T2loc, table_dep=coll1)
        coll2 = nc.gpsimd.collective_compute(
            "AllGather", mybir.AluOpType.bypass,
            replica_groups=[list(range(NCORES))],
            ins=[T3loc[:, :]], outs=[T3[:, :]])
        for d in wd2:
            add_dep_helper(coll2.ins, d.ins, True, "allgather waits T3loc")
        tv3 = [T3[gg * CHUNK:(gg + 1) * CHUNK, :] for gg in range(NCHUNK)]
        run_layer(3, tv3, T3loc, table_dep=coll2)

        guard = cpool.tile([128, H], bf16, tag="guard")
        pw = nc.gpsimd.dma_start(out=guard[:], in_=T2[:128, :H])
        for gth in all_gathers[-8:]:
            add_dep_helper(pw.ins, gth.ins, True, "pool drain guard")
        _gstk.close()
    nc.compile()
    return nc


def kernel(x, edge_index, W1, b1, W2, b2, W3, b3):
    x = np.asarray(x, np.float32)
    edge_index = np.asarray(edge_index, np.int64)
    in_maps, meta = _prep(x, edge_index, W3)
    key = hash(meta)
    if key not in _prog_cache:
        _prog_cache[key] = _build(meta)
    nc = _prog_cache[key]
    shared = {
        "W1": np.asarray(W1, np.float32),
        "b1c": np.asarray(b1, np.float32).reshape(H, 1),
        "W2": np.asarray(W2, np.float32),
        "b2c": np.asarray(b2, np.float32).reshape(H, 1),
        "W3": np.asarray(W3, np.float32).reshape(H, 1),
        "b3r": np.full((128, 1), np.float32(np.asarray(b3).reshape(-1)[0])),
    }
    for m in in_maps:
        m.update(shared)
    res = run_bass_kernel_spmd(nc, in_maps, core_ids=list(range(NCORES)))
    outp = np.concatenate([res.results[c]["out"].reshape(-1)
                           for c in range(NCORES)])
    return outp.reshape(N_NODES, 1).astype(np.float32)
